# revision 31
# baseline (speedup 1.0000x reference)
"""Trainium2 Bass kernel for nn_Clip_OCR_Block (OCR attention block).

Sharding: 8 cores; core j handles image n=j//2, spatial half h=j%2
(8192 of 16384 pixels). The SpatialTemporalGather proxy needs a
full-image spatial reduction -> each core computes partial proxy
numerator/denominator over its half and pair-AllReduces with its
sibling core. Everything else is pixel-local.

v5 structure (vs v4 at 312us):
  - feats ship from the host in BOTH layouts: channel-major bf16
    (q-chain + final conv moving operands) AND spatial-major bf16
    (the proxy's F^T operand). This deletes all 256 PE transposes +
    64 PSUM casts: v4's LDW+MM transpose stream ran at 1.2GHz (HAM
    treats the low-duty stream as idle) and its PSUM-pool coupling
    made the scheduler defer the whole proxy path to ~118us.
  - probs ship pre-transposed [s, k] (pad col 19 = -100 -> exp = 0),
    so eT = one ACT exp. The softmax denominator comes from a DVE
    reduce over a small second read of probs in [k, s] layout.
  - The AllReduce kicks as soon as the 64th proxy matmul retires
    (~30us, gated only by the featsS DMA stream); featsT streams
    behind featsS on the same queue so it cannot steal bandwidth.
  - B2: f_up folded into attention (M = WU @ val^T once; ctx2 =
    relu(M @ sim + bu), contract-19) and the attention chain runs 4
    tiles ahead of the ctx2/final stream (v4: 244ns/MM issue rate).

Matmuls are bf16 except a few tiny f32r ones; bf16 paths sit behind a
softmax or a single layer from the output (v4 measured 2.6e-3 rel-err
vs the 2e-2 gate). BN scales are folded into weights/biases on the
host. Softmaxes skip max-subtraction: |probs| <= ~5.5 and attention
logits are in [0.13, 0.58] for this problem's input distribution.
"""
import numpy as np
import ml_dtypes

import concourse.bacc as bacc
import concourse.mybir as mybir
import concourse.tile as tile
from concourse.bass_utils import run_bass_kernel_spmd

f32 = mybir.dt.float32
f32r = mybir.dt.float32r
bf16 = mybir.dt.bfloat16
AF = mybir.ActivationFunctionType
AX = mybir.AxisListType

N, C, H, W = 4, 512, 128, 128
K, KC, OUT = 19, 256, 512
HW = H * W
HALF = HW // 2            # 8192 pixels per core
NCH = HALF // 128         # 64 chunks of 128 px
NT = HALF // 512          # 16 s-tiles of 512 px
TS = 4 * 512              # elements per F tile per partition
SCALE = KC ** -0.5
KP = 20                   # K padded (junk col 19: exp(-100) = 0)
FS = 16                   # featsS pieces (4 chunks of 128 px each)
AH = 4                    # B2 attention pipeline depth (tiles ahead)

_CACHED = {}


def _build_nc():
    nc = bacc.Bacc("TRN2", target_bir_lowering=False, debug=False, num_devices=8)

    featsT_d = nc.dram_tensor("featsT", [128, NT * TS], bf16, kind="ExternalInput")
    featsS_d = nc.dram_tensor("featsS", [128, NCH * 512], bf16, kind="ExternalInput")
    probsT_d = nc.dram_tensor("probsT", [128, NCH * KP], f32, kind="ExternalInput")
    wp1_d = nc.dram_tensor("wp1b", [128, 4 * 2 * 128], bf16, kind="ExternalInput")
    wp2_d = nc.dram_tensor("wp2b", [128, 2 * 2 * 128], bf16, kind="ExternalInput")
    wo1_d = nc.dram_tensor("wo1b", [128, 4 * 2 * 128], bf16, kind="ExternalInput")
    wo2_d = nc.dram_tensor("wo2b", [128, 2 * 2 * 128], bf16, kind="ExternalInput")
    wd_d = nc.dram_tensor("wdb", [128, 4 * 2 * 128], bf16, kind="ExternalInput")
    wu_d = nc.dram_tensor("wub", [128, 2 * 4 * 128], bf16, kind="ExternalInput")
    wf_d = nc.dram_tensor("wfb", [128, 8 * 4 * 128], bf16, kind="ExternalInput")
    bp1_d = nc.dram_tensor("bp1", [128, 2], f32, kind="ExternalInput")
    bp2_d = nc.dram_tensor("bp2", [128, 2], f32, kind="ExternalInput")
    bo1_d = nc.dram_tensor("bo1", [128, 2], f32, kind="ExternalInput")
    bo2_d = nc.dram_tensor("bo2", [128, 2], f32, kind="ExternalInput")
    bd_d = nc.dram_tensor("bd", [128, 2], f32, kind="ExternalInput")
    bu_d = nc.dram_tensor("bu", [128, 4], f32, kind="ExternalInput")
    bf_d = nc.dram_tensor("bf", [128, 4], f32, kind="ExternalInput")
    ident_d = nc.dram_tensor("ident", [128, 128], f32, kind="ExternalInput")
    ones_d = nc.dram_tensor("ones", [128, 32], f32, kind="ExternalInput")
    out_d = nc.dram_tensor("outT", [128, NT * TS], bf16, kind="ExternalOutput")

    prox_in = nc.dram_tensor("prox_in", [K, C + 1], f32)
    prox_out = nc.dram_tensor("prox_out", [K, C + 1], f32)

    with tile.TileContext(nc) as tc:
        with nc.allow_low_precision(reason="bf16 paths sit behind softmax or one layer from out"), \
             tc.tile_pool(name="w", bufs=1) as wp, \
             tc.tile_pool(name="a", bufs=2) as ap_, \
             tc.tile_pool(name="b", bufs=2) as bp, \
             tc.tile_pool(name="psA", bufs=1, space="PSUM") as ppA, \
             tc.tile_pool(name="psT", bufs=2, space="PSUM") as ppT, \
             tc.tile_pool(name="psM", bufs=4, space="PSUM") as ppM:

            # ---- persistent consts + A0, at the very front of every queue
            # (explicit negative priorities: plain high_priority() restores
            # the counter on exit, so a long block's priorities collide with
            # the instructions emitted after it -- that let q-chain matmuls
            # beat the proxy stream to the PE in v5b)
            with tc.high_priority(offset=200000):
                ident = wp.tile([128, 128], f32r, tag="ident")
                nc.sync.dma_start(ident[:], ident_d.ap().bitcast(f32r))
                ones = wp.tile([128, 32], f32r, tag="ones")
                nc.sync.dma_start(ones[:], ones_d.ap().bitcast(f32r))
                identb = wp.tile([128, 128], bf16, tag="identb")
                nc.vector.tensor_copy(identb[:], ident[:].bitcast(f32))
                onesb = wp.tile([128, 32], bf16, tag="onesb")
                nc.vector.tensor_copy(onesb[:], ones[:].bitcast(f32))

                # PE warmup: dummy matmuls ramp the HAM clock gate while the
                # first DMAs land
                for i in range(24):
                    ps_w = ppM.tile([128, 512], f32, tag="mm", name="ps_warm")
                    nc.tensor.matmul(ps_w[:, :128], ident[:], ident[:],
                                     start=True, stop=True)

            def wload(dram, kin, kout, tag, eng):
                t = wp.tile([128, kin, kout, 128], bf16, tag=tag)
                eng.dma_start(t[:], dram.ap().rearrange(
                    "p (k o m) -> p k o m", k=kin, o=kout))
                return t

            def bload(dram, nch, tag, eng):
                t = wp.tile([128, nch], f32, tag=tag)
                eng.dma_start(t[:], dram.ap())
                return t

            # ---- resident activations ----
            Fres = wp.tile([128, NT, TS], bf16, tag="Fres")
            q2res = wp.tile([128, 2, HALF], bf16, tag="q2res")

            # ========== A0: eT = exp(probsT), in 4 pieces so the first proxy
            # matmuls are not gated on the whole exp ====
            with tc.high_priority(offset=200000):
                pT = wp.tile([128, NCH * KP], f32, tag="pT")
                eT = wp.tile([128, NCH, KP], bf16, tag="eT")
                eTf = eT[:].rearrange("p a k -> p (a k)")
                EP = NCH * KP // 4
                for j in range(4):
                    nc.scalar.dma_start(pT[:, j * EP:(j + 1) * EP],
                                        probsT_d[:, j * EP:(j + 1) * EP])
                    nc.scalar.activation(eTf[:, j * EP:(j + 1) * EP],
                                         pT[:, j * EP:(j + 1) * EP], AF.Exp)

            # early weights (q-chain) on the ACT queue; B2 weights on the idle
            # gpsimd queue so descgen competes with nothing
            wp1 = wload(wp1_d, 4, 2, "wp1", eng=nc.scalar)
            wp2 = wload(wp2_d, 2, 2, "wp2", eng=nc.scalar)
            bp1 = bload(bp1_d, 2, "bp1", eng=nc.scalar)
            bp2 = bload(bp2_d, 2, "bp2", eng=nc.scalar)

            # ============ B1: proxy = eT^T @ F^T, fed by the featsS stream ====
            # high_priority pins the proxy path at the front of the PE stream
            # (the list scheduler otherwise interleaves q-chain matmuls ahead
            # of it, delaying the AllReduce kick by ~70us). den rides the same
            # eT stationary as the proxy.
            ps_prox = ppA.tile([KP, C], f32, tag="prox")
            ps_den = ppA.tile([KP, 32], f32, tag="den")
            with tc.high_priority(offset=100000):
                for pc in range(FS):
                    fS = ap_.tile([128, 4, 512], bf16, tag="fS", bufs=6)
                    nc.sync.dma_start(
                        fS[:], featsS_d[:, pc * 2048:(pc + 1) * 2048].rearrange(
                            "p (c s) -> p c s", c=4))
                    for c in range(4):
                        tt = pc * 4 + c
                        nc.tensor.matmul(ps_prox[:], eT[:, tt, :], fS[:, c, :],
                                         start=(tt == 0), stop=(tt == NCH - 1))
                        nc.tensor.matmul(ps_den[:], eT[:, tt, :], onesb[:],
                                         start=(tt == 0), stop=(tt == NCH - 1))

            # B2 weights on the idle gpsimd queue, AFTER B1 in program order so
            # their 2.4MB does not steal HBM bandwidth from the featsS stream
            wo1 = wload(wo1_d, 4, 2, "wo1", eng=nc.gpsimd)
            wo2 = wload(wo2_d, 2, 2, "wo2", eng=nc.gpsimd)
            wd = wload(wd_d, 4, 2, "wd", eng=nc.gpsimd)
            wu = wload(wu_d, 2, 4, "wu", eng=nc.gpsimd)
            wf = wload(wf_d, 8, 4, "wf", eng=nc.gpsimd)
            bo1 = bload(bo1_d, 2, "bo1", eng=nc.gpsimd)
            bo2 = bload(bo2_d, 2, "bo2", eng=nc.gpsimd)
            bd = bload(bd_d, 2, "bd", eng=nc.gpsimd)
            bu = bload(bu_d, 4, "bu", eng=nc.gpsimd)
            bf = bload(bf_d, 4, "bf", eng=nc.gpsimd)

            # ============ AllReduce proxy partials with pair core ============
            # (program order places these right after B1 on the DVE/gpsimd
            # queues; gpsimd is idle so the waiting prox_in DMA blocks nothing)
            prox_sb = wp.tile([K, C + 1], f32, tag="proxsb")
            nc.vector.tensor_copy(prox_sb[:, 1:], ps_prox[:K, :])
            nc.vector.tensor_copy(prox_sb[:, 0:1], ps_den[:K, 0:1])
            nc.gpsimd.dma_start(prox_in[:], prox_sb[:])
            nc.gpsimd.collective_compute(
                "AllReduce", mybir.AluOpType.add,
                replica_groups=[[0, 1], [2, 3], [4, 5], [6, 7]],
                ins=[prox_in[:]], outs=[prox_out[:]])

            # featsT streams behind featsS on the same queue
            for t in range(NT):
                nc.sync.dma_start(Fres[:, t, :],
                                  featsT_d[:, t * TS:(t + 1) * TS])

            # ---- q-chains: q2 = cbr(cbr(F, p1), p2), resident bf16 ----
            for t in range(NT):
                q1 = bp.tile([128, 2, 512], bf16, tag="q1", name="q1")
                for o in range(2):
                    ps = ppM.tile([128, 512], f32, tag="mm", name="ps_q1")
                    for k in range(4):
                        nc.tensor.matmul(ps[:], wp1[:, k, o, :],
                                         Fres[:, t, k * 512:(k + 1) * 512],
                                         start=(k == 0), stop=(k == 3))
                    nc.vector.tensor_scalar(q1[:, o, :], ps[:], bp1[:, o:o + 1],
                                            0.0, mybir.AluOpType.add,
                                            mybir.AluOpType.max)
                for o in range(2):
                    ps = ppM.tile([128, 512], f32, tag="mm", name="ps_q2")
                    for k in range(2):
                        nc.tensor.matmul(ps[:], wp2[:, k, o, :], q1[:, k, :],
                                         start=(k == 0), stop=(k == 1))
                    nc.scalar.activation(q2res[:, o, t * 512:(t + 1) * 512], ps[:],
                                         AF.Relu, bias=bp2[:, o:o + 1], scale=1.0)

            # ---- post-collective smalls: proxy -> kk, M = WU @ val^T ----
            red = wp.tile([K, C + 1], f32, tag="red")
            nc.gpsimd.dma_start(red[:], prox_out[:])
            recip = wp.tile([K, 1], f32, tag="recip")
            nc.vector.reciprocal(recip[:], red[:, 0:1])
            prox_n = wp.tile([K, C], f32r, tag="proxn")
            nc.vector.tensor_scalar_mul(prox_n[:], in0=red[:, 1:], scalar1=recip[:])

            # proxy -> [c, k] layout (LDW+MM transposes)
            proxT = wp.tile([128, 4, KP], bf16, tag="proxT")
            for a in range(4):
                ps_t = ppT.tile([128, 128], f32, tag="tr", name="ps_tr2")
                nc.tensor.matmul(ps_t[:, :KP], prox_n[:, a * 128:(a + 1) * 128],
                                 ident[:K, :KP], start=True, stop=True)
                nc.vector.tensor_copy(proxT[:, a, :], ps_t[:, :KP])

            def small_conv(wt, bt, rhs_tile, kin, kout, tag):
                res = wp.tile([128, kout, KP], bf16, tag=tag)
                for o in range(kout):
                    ps = ppM.tile([128, 512], f32, tag="mm", name="ps_sc")
                    ps = ps[:, :KP]
                    for k in range(kin):
                        nc.tensor.matmul(ps[:], wt[:, k, o, :], rhs_tile[:, k, :],
                                         start=(k == 0), stop=(k == kin - 1))
                    nc.scalar.activation(res[:, o, :], ps[:], AF.Relu,
                                         bias=bt[:, o:o + 1], scale=1.0)
                return res

            kk1 = small_conv(wo1, bo1, proxT, 4, 2, "kk1")
            kk = small_conv(wo2, bo2, kk1, 2, 2, "kk")
            val_cb = small_conv(wd, bd, proxT, 4, 2, "valcb")

            # M^T[k, c] = sum_kc val^T[kc, k] * WU^T[kc, c]  ([19, 512])
            psMT = ppT.tile([KP, 512], f32, tag="tr", name="ps_mt")
            for o in range(4):
                for k in range(2):
                    nc.tensor.matmul(psMT[:, o * 128:(o + 1) * 128],
                                     val_cb[:, k, :], wu[:, k, o, :],
                                     start=(k == 0), stop=(k == 1))
            # 4-replica tiles for the row-tiled B2: the 4 contract-19 ctx2
            # matmuls issue to distinct 32-row groups (tile_position) and
            # distinct PSUM banks, so they run concurrently on the PE's
            # 32x32 sub-arrays instead of serially.
            MT4 = wp.tile([128, 512], bf16, tag="MT4")
            for j in range(4):
                nc.vector.tensor_copy(MT4[32 * j:32 * j + K, :], psMT[:K, :])
            kk4 = wp.tile([128, 2, 128], bf16, tag="kk4")
            nc.vector.memset(kk4[:], 0.0)
            for k in range(2):
                for j in range(4):
                    nc.vector.tensor_copy(kk4[:, k, 32 * j:32 * j + K],
                                          kk[:, k, 0:K])
            bcmask = wp.tile([1, 128], bf16, tag="bcmask")
            nc.vector.memset(bcmask[:], 0.0)
            for j in range(4):
                nc.vector.tensor_copy(bcmask[0:1, 32 * j:32 * j + K],
                                      onesb[0:1, 0:K])
            denmask = wp.tile([128, 1], bf16, tag="denmask")
            nc.vector.memset(denmask[:], 0.0)
            nc.vector.tensor_copy(denmask[0:K, :], onesb[0:K, 0:1])

            # ============ B2: attention + folded f_up + final conv ============
            # The attention chain for tile t+AH runs interleaved with tile t's
            # ctx2/final stream, so exp/recip/mul latencies are hidden behind
            # ~8us of dense matmul work.
            st = [dict() for _ in range(NT)]

            def attA(t):   # logits + exp, replicated to 4 row groups
                d = st[t]
                ps_log = ppT.tile([128, 512], f32, tag="tr", name="ps_log")
                for k in range(2):
                    nc.tensor.matmul(ps_log[:], kk4[:, k, :],
                                     q2res[:, k, t * 512:(t + 1) * 512],
                                     start=(k == 0), stop=(k == 1))
                e_att = bp.tile([128, 512], bf16, tag="eatt", bufs=3,
                                name="e_att")
                nc.scalar.activation(e_att[:], ps_log[:], AF.Exp, scale=SCALE)
                d["e_att"] = e_att

            def attB(t):   # denominator + reciprocal (masked row sum)
                d = st[t]
                ps_dn = ppT.tile([128, 512], f32, tag="tr", name="ps_dn")
                nc.tensor.matmul(ps_dn[:1, :], denmask[:], d["e_att"][:],
                                 start=True, stop=True)
                rc32 = bp.tile([1, 512], f32, tag="rc32", name="rc32")
                nc.vector.reciprocal_approx_fast(rc32[:], ps_dn[:1, :])
                rc = bp.tile([1, 512], bf16, tag="rc", name="rc")
                nc.scalar.activation(rc[:], rc32[:], AF.Copy)
                d["rc"] = rc

            def attC(t):   # broadcast 1/den to the 4 row groups (0 elsewhere,
                d = st[t]  # which zeroes e_att's exp(0)=1 junk rows in sim)
                ps_bc = ppT.tile([128, 512], f32, tag="tr", name="ps_bc")
                nc.tensor.matmul(ps_bc[:], bcmask[:], d["rc"][:],
                                 start=True, stop=True)
                d["ps_bc"] = ps_bc

            def attD(t):   # sim = e_att * (1/den)
                d = st[t]
                sim = bp.tile([128, 512], bf16, tag="sim", bufs=AH + 2,
                              name="sim")
                nc.vector.tensor_mul(sim[:], d["e_att"][:], d["ps_bc"][:])
                d["sim"] = sim

            def ctx2f(t, orange):
                d = st[t]
                if "ctx2" not in d:
                    d["ctx2"] = bp.tile([128, 4, 512], bf16, tag="ctx2",
                                        name="ctx2")
                for o in orange:
                    ps = ppM.tile([128, 512], f32, tag="mm")
                    nc.tensor.matmul(ps[:],
                                     MT4[32 * o:32 * o + K,
                                         o * 128:(o + 1) * 128],
                                     d["sim"][32 * o:32 * o + K, :],
                                     start=True, stop=True,
                                     tile_position=(32 * o, 0))
                    # relu(x + b): split between DVE and ACT to balance B2
                    if o % 2 == 0:
                        nc.vector.tensor_scalar(d["ctx2"][:, o, :], ps[:],
                                                bu[:, o:o + 1], 0.0,
                                                mybir.AluOpType.add,
                                                mybir.AluOpType.max)
                    else:
                        nc.scalar.activation(d["ctx2"][:, o, :], ps[:], AF.Relu,
                                             bias=bu[:, o:o + 1], scale=1.0)

            def final(t, orange):
                d = st[t]
                if "ot" not in d:
                    d["ot"] = bp.tile([128, 4, 512], bf16, tag="out", bufs=2,
                                      name="ot")
                ot = d["ot"]
                korder = [4, 5, 6, 7, 0, 1, 2, 3]
                for o in orange:
                    ps = ppM.tile([128, 512], f32, tag="mm")
                    for i, k in enumerate(korder):
                        rhs = (d["ctx2"][:, k, :] if k < 4
                               else Fres[:, t, (k - 4) * 512:(k - 3) * 512])
                        nc.tensor.matmul(ps[:], wf[:, k, o, :], rhs,
                                         start=(i == 0), stop=(i == 7))
                    if o % 2 == 0:
                        nc.vector.tensor_scalar(ot[:, o, :], ps[:],
                                                bf[:, o:o + 1], 0.0,
                                                mybir.AluOpType.add,
                                                mybir.AluOpType.max)
                    else:
                        nc.scalar.activation(ot[:, o, :], ps[:], AF.Relu,
                                             bias=bf[:, o:o + 1], scale=1.0)
                if orange[-1] == 3:
                    nc.sync.dma_start(
                        out_d[:, t * TS:(t + 1) * TS],
                        ot[:].rearrange("p a s -> p (a s)"))
                    st[t] = None

            # attention ramp for tiles 0..AH-1 (wavefront order)
            stages = [attA, attB, attC, attD]
            for s in range(AH + 3):
                for k, f in enumerate(stages):
                    t = s - k
                    if 0 <= t < AH:
                        f(t)

            # per-step order: the exp/recip latencies after attA/attB need
            # >1us of PE work between the attention stages -- the final-conv
            # chains provide it (attB only ~0.5us after attA stalled ~0.7us
            # per tile in earlier layouts)
            for t in range(NT):
                ta = t + AH
                if ta < NT:
                    attA(ta)
                ctx2f(t, (0, 1))
                ctx2f(t, (2, 3))
                final(t, (0, 1))
                if ta < NT:
                    attB(ta)
                final(t, (2,))
                if ta < NT:
                    attC(ta)
                final(t, (3,))
                if ta < NT:
                    attD(ta)

    nc.compile()
    return nc


def _fold(w, b, s, t):
    """conv+BN fold: y = s*(Wx+b)+t = (s.W)x + (s*b+t)."""
    w = np.asarray(w, np.float32)
    b = np.asarray(b, np.float32)
    s = np.asarray(s, np.float32)
    t = np.asarray(t, np.float32)
    return (s[:, None] * w), (s * b + t)


def _tw(Wmat, kin, kout):
    """[out, in] f32 -> pre-tiled [128, kin*kout*128] bf16 (stationary tiles
    [in-chunk, out-chunk] contiguous per partition)."""
    Wt = np.asarray(Wmat, np.float32).T  # [in, out]
    t = Wt.reshape(kin, 128, kout, 128).transpose(1, 0, 2, 3)
    return np.ascontiguousarray(t.reshape(128, kin * kout * 128)
                                .astype(ml_dtypes.bfloat16))


def _tb(b, nch):
    """[nch*128] f32 -> [128, nch] (partition-major bias tiles)."""
    return np.ascontiguousarray(np.asarray(b, np.float32).reshape(nch, 128).T)


def kernel(feats, probs,
           wp1, bp1, sp1, tp1, wp2, bp2, sp2, tp2,
           wo1, bo1, so1, to1, wo2, bo2, so2, to2,
           wd, bd, sd, td, wu, bu, su, tu,
           wf, bf, sf, tf, clip_num, _trace=False):
    feats = np.asarray(feats, np.float32)
    probs = np.ascontiguousarray(np.asarray(probs, np.float32))

    W1, B1 = _fold(wp1, bp1, sp1, tp1)
    W2, B2 = _fold(wp2, bp2, sp2, tp2)
    WO1, BO1 = _fold(wo1, bo1, so1, to1)
    WO2, BO2 = _fold(wo2, bo2, so2, to2)
    WD, BD = _fold(wd, bd, sd, td)
    WU, BU = _fold(wu, bu, su, tu)
    WF, BF = _fold(wf, bf, sf, tf)

    shared = {
        "wp1b": _tw(W1, 4, 2), "bp1": _tb(B1, 2),
        "wp2b": _tw(W2, 2, 2), "bp2": _tb(B2, 2),
        "wo1b": _tw(WO1, 4, 2), "bo1": _tb(BO1, 2),
        "wo2b": _tw(WO2, 2, 2), "bo2": _tb(BO2, 2),
        "wdb": _tw(WD, 4, 2), "bd": _tb(BD, 2),
        "wub": _tw(WU, 2, 4), "bu": _tb(BU, 4),
        "wfb": _tw(WF, 8, 4), "bf": _tb(BF, 4),
        "ident": np.eye(128, dtype=np.float32),
        "ones": np.ones((128, 32), np.float32),
    }

    fb = np.asarray(feats.reshape(N, C, HW), np.float32).astype(ml_dtypes.bfloat16)
    pr = probs.reshape(N, K, HW)
    in_maps = []
    for j in range(8):
        n, h = j // 2, j % 2
        sl = slice(h * HALF, (h + 1) * HALF)
        fh = fb[n, :, sl]
        # channel-major tiles: (p, t, a, s) = F[a*128+p, t*512+s]
        fT = fh.reshape(4, 128, NT, 512).transpose(1, 2, 0, 3)
        # spatial-major (pre-transposed): (p, tt, c) = F[c, tt*128+p]
        fS = np.ascontiguousarray(fh.T).reshape(NCH, 128, C).transpose(1, 0, 2)
        # probsT: (p, tt, k) = probs[k, tt*128+p], col 19 = -100 -> exp = 0
        ph = pr[n, :, sl]
        pt = np.full((HALF, KP), -100.0, np.float32)
        pt[:, :K] = ph.T
        pt = pt.reshape(NCH, 128, KP).transpose(1, 0, 2)
        in_maps.append({
            "featsT": np.ascontiguousarray(fT.reshape(128, NT * TS)),
            "featsS": np.ascontiguousarray(fS.reshape(128, NCH * C)),
            "probsT": np.ascontiguousarray(pt.reshape(128, NCH * KP)),
            **shared,
        })

    if "nc" not in _CACHED:
        _CACHED["nc"] = _build_nc()
    nc = _CACHED["nc"]

    res = run_bass_kernel_spmd(nc, in_maps, list(range(8)), trace=_trace)
    out = np.empty((N, OUT, HW), np.float32)
    for j in range(8):
        n, h = j // 2, j % 2
        o = np.asarray(res.results[j]["outT"], np.float32).reshape(128, NT, 4, 512)
        out[n, :, h * HALF:(h + 1) * HALF] = (
            o.transpose(2, 0, 1, 3).reshape(OUT, HALF))
    if _trace:
        kernel.last_exec_time_ns = res.exec_time_ns
        kernel.last_results = res
    return out.reshape(N, OUT, H, W)


# revision 32
# speedup vs baseline: 1.0142x; 1.0142x over previous
"""Trainium2 Bass kernel for nn_Clip_OCR_Block (OCR attention block).

Sharding: 8 cores; core j handles image n=j//2, spatial half h=j%2
(8192 of 16384 pixels). The SpatialTemporalGather proxy needs a
full-image spatial reduction -> each core computes partial proxy
numerator/denominator over its half and pair-AllReduces with its
sibling core. Everything else is pixel-local.

v5 structure (vs v4 at 312us):
  - feats ship from the host in BOTH layouts: channel-major bf16
    (q-chain + final conv moving operands) AND spatial-major bf16
    (the proxy's F^T operand). This deletes all 256 PE transposes +
    64 PSUM casts: v4's LDW+MM transpose stream ran at 1.2GHz (HAM
    treats the low-duty stream as idle) and its PSUM-pool coupling
    made the scheduler defer the whole proxy path to ~118us.
  - probs ship pre-transposed [s, k] (pad col 19 = -100 -> exp = 0),
    so eT = one ACT exp. The softmax denominator comes from a DVE
    reduce over a small second read of probs in [k, s] layout.
  - The AllReduce kicks as soon as the 64th proxy matmul retires
    (~30us, gated only by the featsS DMA stream); featsT streams
    behind featsS on the same queue so it cannot steal bandwidth.
  - B2: f_up folded into attention (M = WU @ val^T once; ctx2 =
    relu(M @ sim + bu), contract-19) and the attention chain runs 4
    tiles ahead of the ctx2/final stream (v4: 244ns/MM issue rate).

Matmuls are bf16 except a few tiny f32r ones; bf16 paths sit behind a
softmax or a single layer from the output (v4 measured 2.6e-3 rel-err
vs the 2e-2 gate). BN scales are folded into weights/biases on the
host. Softmaxes skip max-subtraction: |probs| <= ~5.5 and attention
logits are in [0.13, 0.58] for this problem's input distribution.
"""
import numpy as np
import ml_dtypes

import concourse.bacc as bacc
import concourse.mybir as mybir
import concourse.tile as tile
from concourse.bass_utils import run_bass_kernel_spmd

f32 = mybir.dt.float32
f32r = mybir.dt.float32r
bf16 = mybir.dt.bfloat16
AF = mybir.ActivationFunctionType
AX = mybir.AxisListType

N, C, H, W = 4, 512, 128, 128
K, KC, OUT = 19, 256, 512
HW = H * W
HALF = HW // 2            # 8192 pixels per core
NCH = HALF // 128         # 64 chunks of 128 px
NT = HALF // 512          # 16 s-tiles of 512 px
TS = 4 * 512              # elements per F tile per partition
SCALE = KC ** -0.5
KP = 20                   # K padded (junk col 19: exp(-100) = 0)
FS = 16                   # featsS pieces (4 chunks of 128 px each)
AH = 4                    # B2 attention pipeline depth (tiles ahead)

_CACHED = {}


def _build_nc():
    nc = bacc.Bacc("TRN2", target_bir_lowering=False, debug=False, num_devices=8)

    featsT_d = nc.dram_tensor("featsT", [128, NT * TS], bf16, kind="ExternalInput")
    featsS_d = nc.dram_tensor("featsS", [128, NCH * 512], bf16, kind="ExternalInput")
    probsT_d = nc.dram_tensor("probsT", [128, NCH * KP], f32, kind="ExternalInput")
    wp1_d = nc.dram_tensor("wp1b", [128, 4 * 2 * 128], bf16, kind="ExternalInput")
    wp2_d = nc.dram_tensor("wp2b", [128, 2 * 2 * 128], bf16, kind="ExternalInput")
    wo1_d = nc.dram_tensor("wo1b", [128, 4 * 2 * 128], bf16, kind="ExternalInput")
    wo2_d = nc.dram_tensor("wo2b", [128, 2 * 2 * 128], bf16, kind="ExternalInput")
    wd_d = nc.dram_tensor("wdb", [128, 4 * 2 * 128], bf16, kind="ExternalInput")
    wu_d = nc.dram_tensor("wub", [128, 2 * 4 * 128], bf16, kind="ExternalInput")
    wf_d = nc.dram_tensor("wfb", [128, 8 * 4 * 128], bf16, kind="ExternalInput")
    bp1_d = nc.dram_tensor("bp1", [128, 2], f32, kind="ExternalInput")
    bp2_d = nc.dram_tensor("bp2", [128, 2], f32, kind="ExternalInput")
    bo1_d = nc.dram_tensor("bo1", [128, 2], f32, kind="ExternalInput")
    bo2_d = nc.dram_tensor("bo2", [128, 2], f32, kind="ExternalInput")
    bd_d = nc.dram_tensor("bd", [128, 2], f32, kind="ExternalInput")
    bu_d = nc.dram_tensor("bu", [128, 4], f32, kind="ExternalInput")
    bf_d = nc.dram_tensor("bf", [128, 4], f32, kind="ExternalInput")
    ident_d = nc.dram_tensor("ident", [128, 128], f32, kind="ExternalInput")
    ones_d = nc.dram_tensor("ones", [128, 32], f32, kind="ExternalInput")
    out_d = nc.dram_tensor("outT", [128, NT * TS], bf16, kind="ExternalOutput")

    prox_in = nc.dram_tensor("prox_in", [K, C + 1], f32)
    prox_out = nc.dram_tensor("prox_out", [K, C + 1], f32)

    with tile.TileContext(nc) as tc:
        with nc.allow_low_precision(reason="bf16 paths sit behind softmax or one layer from out"), \
             tc.tile_pool(name="w", bufs=1) as wp, \
             tc.tile_pool(name="a", bufs=2) as ap_, \
             tc.tile_pool(name="b", bufs=2) as bp, \
             tc.tile_pool(name="psA", bufs=1, space="PSUM") as ppA, \
             tc.tile_pool(name="psT", bufs=2, space="PSUM") as ppT, \
             tc.tile_pool(name="psM", bufs=4, space="PSUM") as ppM:

            # ---- persistent consts + A0, at the very front of every queue
            # (explicit negative priorities: plain high_priority() restores
            # the counter on exit, so a long block's priorities collide with
            # the instructions emitted after it -- that let q-chain matmuls
            # beat the proxy stream to the PE in v5b)
            with tc.high_priority(offset=200000):
                ident = wp.tile([128, 128], f32r, tag="ident")
                nc.sync.dma_start(ident[:], ident_d.ap().bitcast(f32r))
                ones = wp.tile([128, 32], f32r, tag="ones")
                nc.sync.dma_start(ones[:], ones_d.ap().bitcast(f32r))
                identb = wp.tile([128, 128], bf16, tag="identb")
                nc.vector.tensor_copy(identb[:], ident[:].bitcast(f32))
                onesb = wp.tile([128, 32], bf16, tag="onesb")
                nc.vector.tensor_copy(onesb[:], ones[:].bitcast(f32))

                # PE warmup: dummy matmuls ramp the HAM clock gate while the
                # first DMAs land
                for i in range(24):
                    ps_w = ppM.tile([128, 512], f32, tag="mm", name="ps_warm")
                    nc.tensor.matmul(ps_w[:, :128], ident[:], ident[:],
                                     start=True, stop=True)

            def wload(dram, kin, kout, tag, eng):
                t = wp.tile([128, kin, kout, 128], bf16, tag=tag)
                eng.dma_start(t[:], dram.ap().rearrange(
                    "p (k o m) -> p k o m", k=kin, o=kout))
                return t

            def bload(dram, nch, tag, eng):
                t = wp.tile([128, nch], f32, tag=tag)
                eng.dma_start(t[:], dram.ap())
                return t

            # ---- resident activations ----
            Fres = wp.tile([128, NT, TS], bf16, tag="Fres")
            q2res = wp.tile([128, 2, HALF], bf16, tag="q2res")

            # ========== A0: eT = exp(probsT), in 4 pieces so the first proxy
            # matmuls are not gated on the whole exp ====
            with tc.high_priority(offset=200000):
                pT = wp.tile([128, NCH * KP], f32, tag="pT")
                eT = wp.tile([128, NCH, KP], bf16, tag="eT")
                eTf = eT[:].rearrange("p a k -> p (a k)")
                EP = NCH * KP // 4
                for j in range(4):
                    nc.scalar.dma_start(pT[:, j * EP:(j + 1) * EP],
                                        probsT_d[:, j * EP:(j + 1) * EP])
                    nc.scalar.activation(eTf[:, j * EP:(j + 1) * EP],
                                         pT[:, j * EP:(j + 1) * EP], AF.Exp)

            # early weights (q-chain) on the ACT queue; B2 weights on the idle
            # gpsimd queue so descgen competes with nothing
            wp1 = wload(wp1_d, 4, 2, "wp1", eng=nc.scalar)
            wp2 = wload(wp2_d, 2, 2, "wp2", eng=nc.scalar)
            bp1 = bload(bp1_d, 2, "bp1", eng=nc.scalar)
            bp2 = bload(bp2_d, 2, "bp2", eng=nc.scalar)

            # mask tiles for the row-tiled B2 (dependency-free: built early)
            bcmask = wp.tile([1, 128], bf16, tag="bcmask")
            nc.vector.memset(bcmask[:], 0.0)
            for j in range(4):
                nc.vector.tensor_copy(bcmask[0:1, 32 * j:32 * j + K],
                                      onesb[0:1, 0:K])
            denmask = wp.tile([128, 1], bf16, tag="denmask")
            nc.vector.memset(denmask[:], 0.0)
            nc.vector.tensor_copy(denmask[0:K, :], onesb[0:K, 0:1])
            kk4 = wp.tile([128, 2, 128], bf16, tag="kk4")
            nc.vector.memset(kk4[:], 0.0)

            # ============ B1: proxy = eT^T @ F^T, fed by the featsS stream ====
            # high_priority pins the proxy path at the front of the PE stream
            # (the list scheduler otherwise interleaves q-chain matmuls ahead
            # of it, delaying the AllReduce kick by ~70us). den rides the same
            # eT stationary as the proxy.
            ps_prox = ppA.tile([KP, C], f32, tag="prox")
            ps_den = ppA.tile([KP, 32], f32, tag="den")
            with tc.high_priority(offset=100000):
                for pc in range(FS):
                    fS = ap_.tile([128, 4, 512], bf16, tag="fS", bufs=6)
                    nc.sync.dma_start(
                        fS[:], featsS_d[:, pc * 2048:(pc + 1) * 2048].rearrange(
                            "p (c s) -> p c s", c=4))
                    for c in range(4):
                        tt = pc * 4 + c
                        nc.tensor.matmul(ps_prox[:], eT[:, tt, :], fS[:, c, :],
                                         start=(tt == 0), stop=(tt == NCH - 1))
                        nc.tensor.matmul(ps_den[:], eT[:, tt, :], onesb[:],
                                         start=(tt == 0), stop=(tt == NCH - 1))

            # B2 weights on the idle gpsimd queue, AFTER B1 in program order so
            # their 2.4MB does not steal HBM bandwidth from the featsS stream
            wo1 = wload(wo1_d, 4, 2, "wo1", eng=nc.gpsimd)
            wo2 = wload(wo2_d, 2, 2, "wo2", eng=nc.gpsimd)
            wd = wload(wd_d, 4, 2, "wd", eng=nc.gpsimd)
            wu = wload(wu_d, 2, 4, "wu", eng=nc.gpsimd)
            wf = wload(wf_d, 8, 4, "wf", eng=nc.gpsimd)
            bo1 = bload(bo1_d, 2, "bo1", eng=nc.gpsimd)
            bo2 = bload(bo2_d, 2, "bo2", eng=nc.gpsimd)
            bd = bload(bd_d, 2, "bd", eng=nc.gpsimd)
            bu = bload(bu_d, 4, "bu", eng=nc.gpsimd)
            bf = bload(bf_d, 4, "bf", eng=nc.gpsimd)

            # ============ AllReduce proxy partials with pair core ============
            # (program order places these right after B1 on the DVE/gpsimd
            # queues; gpsimd is idle so the waiting prox_in DMA blocks nothing)
            prox_sb = wp.tile([K, C + 1], f32, tag="proxsb")
            nc.vector.tensor_copy(prox_sb[:, 1:], ps_prox[:K, :])
            nc.vector.tensor_copy(prox_sb[:, 0:1], ps_den[:K, 0:1])
            nc.gpsimd.dma_start(prox_in[:], prox_sb[:])
            nc.gpsimd.collective_compute(
                "AllReduce", mybir.AluOpType.add,
                replica_groups=[[0, 1], [2, 3], [4, 5], [6, 7]],
                ins=[prox_in[:]], outs=[prox_out[:]])

            # featsT streams behind featsS on the same queue
            for t in range(NT):
                nc.sync.dma_start(Fres[:, t, :],
                                  featsT_d[:, t * TS:(t + 1) * TS])

            # ---- q-chains: q2 = cbr(cbr(F, p1), p2), resident bf16 ----
            for t in range(NT):
                q1 = bp.tile([128, 2, 512], bf16, tag="q1", name="q1")
                for o in range(2):
                    ps = ppM.tile([128, 512], f32, tag="mm", name="ps_q1")
                    for k in range(4):
                        nc.tensor.matmul(ps[:], wp1[:, k, o, :],
                                         Fres[:, t, k * 512:(k + 1) * 512],
                                         start=(k == 0), stop=(k == 3))
                    nc.vector.tensor_scalar(q1[:, o, :], ps[:], bp1[:, o:o + 1],
                                            0.0, mybir.AluOpType.add,
                                            mybir.AluOpType.max)
                for o in range(2):
                    ps = ppM.tile([128, 512], f32, tag="mm", name="ps_q2")
                    for k in range(2):
                        nc.tensor.matmul(ps[:], wp2[:, k, o, :], q1[:, k, :],
                                         start=(k == 0), stop=(k == 1))
                    nc.scalar.activation(q2res[:, o, t * 512:(t + 1) * 512], ps[:],
                                         AF.Relu, bias=bp2[:, o:o + 1], scale=1.0)

            # ---- post-collective smalls: proxy -> kk, M = WU @ val^T ----
            red = wp.tile([K, C + 1], f32, tag="red")
            nc.gpsimd.dma_start(red[:], prox_out[:])
            recip = wp.tile([K, 1], f32, tag="recip")
            nc.vector.reciprocal(recip[:], red[:, 0:1])
            prox_n = wp.tile([K, C], f32r, tag="proxn")
            nc.vector.tensor_scalar_mul(prox_n[:], in0=red[:, 1:], scalar1=recip[:])

            # proxy -> [c, k] layout (LDW+MM transposes)
            proxT = wp.tile([128, 4, KP], bf16, tag="proxT")
            for a in range(4):
                ps_t = ppT.tile([128, 128], f32, tag="tr", name="ps_tr2")
                nc.tensor.matmul(ps_t[:, :KP], prox_n[:, a * 128:(a + 1) * 128],
                                 ident[:K, :KP], start=True, stop=True)
                nc.vector.tensor_copy(proxT[:, a, :], ps_t[:, :KP])

            def small_conv(wt, bt, rhs_tile, kin, kout, tag):
                res = wp.tile([128, kout, KP], bf16, tag=tag)
                for o in range(kout):
                    ps = ppM.tile([128, 512], f32, tag="mm", name="ps_sc")
                    ps = ps[:, :KP]
                    for k in range(kin):
                        nc.tensor.matmul(ps[:], wt[:, k, o, :], rhs_tile[:, k, :],
                                         start=(k == 0), stop=(k == kin - 1))
                    nc.scalar.activation(res[:, o, :], ps[:], AF.Relu,
                                         bias=bt[:, o:o + 1], scale=1.0)
                return res

            kk1 = small_conv(wo1, bo1, proxT, 4, 2, "kk1")
            kk = small_conv(wo2, bo2, kk1, 2, 2, "kk")
            for k in range(2):
                for j in range(4):
                    nc.vector.tensor_copy(kk4[:, k, 32 * j:32 * j + K],
                                          kk[:, k, 0:K])
            val_cb = small_conv(wd, bd, proxT, 4, 2, "valcb")

            # M^T[k, c] = sum_kc val^T[kc, k] * WU^T[kc, c]  ([19, 512])
            psMT = ppT.tile([KP, 512], f32, tag="tr", name="ps_mt")
            for o in range(4):
                for k in range(2):
                    nc.tensor.matmul(psMT[:, o * 128:(o + 1) * 128],
                                     val_cb[:, k, :], wu[:, k, o, :],
                                     start=(k == 0), stop=(k == 1))
            # 4-replica tiles for the row-tiled B2: the 4 contract-19 ctx2
            # matmuls issue to distinct 32-row groups (tile_position) and
            # distinct PSUM banks, so they run concurrently on the PE's
            # 32x32 sub-arrays instead of serially.
            MT4 = wp.tile([128, 512], bf16, tag="MT4")
            for j in range(4):
                nc.vector.tensor_copy(MT4[32 * j:32 * j + K, :], psMT[:K, :])

            for i in range(6):
                ps_w = ppM.tile([128, 512], f32, tag="mm", name="ps_kw")
                nc.tensor.matmul(ps_w[:, :128], ident[:], ident[:],
                                 start=True, stop=True)

            # ============ B2: attention + folded f_up + final conv ============
            # The attention chain for tile t+AH runs interleaved with tile t's
            # ctx2/final stream, so exp/recip/mul latencies are hidden behind
            # ~8us of dense matmul work.
            st = [dict() for _ in range(NT)]

            def attA(t):   # logits + exp, replicated to 4 row groups
                d = st[t]
                ps_log = ppT.tile([128, 512], f32, tag="tr", name="ps_log")
                for k in range(2):
                    nc.tensor.matmul(ps_log[:], kk4[:, k, :],
                                     q2res[:, k, t * 512:(t + 1) * 512],
                                     start=(k == 0), stop=(k == 1))
                e_att = bp.tile([128, 512], bf16, tag="eatt", bufs=3,
                                name="e_att")
                nc.scalar.activation(e_att[:], ps_log[:], AF.Exp, scale=SCALE)
                d["e_att"] = e_att

            def attB(t):   # denominator + reciprocal (masked row sum)
                d = st[t]
                ps_dn = ppT.tile([128, 512], f32, tag="tr", name="ps_dn")
                nc.tensor.matmul(ps_dn[:1, :], denmask[:], d["e_att"][:],
                                 start=True, stop=True)
                rc32 = bp.tile([1, 512], f32, tag="rc32", name="rc32")
                nc.vector.reciprocal_approx_fast(rc32[:], ps_dn[:1, :])
                rc = bp.tile([1, 512], bf16, tag="rc", name="rc")
                nc.scalar.activation(rc[:], rc32[:], AF.Copy)
                d["rc"] = rc

            def attC(t):   # broadcast 1/den to the 4 row groups (0 elsewhere,
                d = st[t]  # which zeroes e_att's exp(0)=1 junk rows in sim)
                ps_bc = ppT.tile([128, 512], f32, tag="tr", name="ps_bc")
                nc.tensor.matmul(ps_bc[:], bcmask[:], d["rc"][:],
                                 start=True, stop=True)
                d["ps_bc"] = ps_bc

            def attD(t):   # sim = e_att * (1/den)
                d = st[t]
                sim = bp.tile([128, 512], bf16, tag="sim", bufs=AH + 2,
                              name="sim")
                nc.vector.tensor_mul(sim[:], d["e_att"][:], d["ps_bc"][:])
                d["sim"] = sim

            def ctx2f(t, orange):
                d = st[t]
                if "ctx2" not in d:
                    d["ctx2"] = bp.tile([128, 4, 512], bf16, tag="ctx2",
                                        name="ctx2")
                for o in orange:
                    ps = ppM.tile([128, 512], f32, tag="mm")
                    nc.tensor.matmul(ps[:],
                                     MT4[32 * o:32 * o + K,
                                         o * 128:(o + 1) * 128],
                                     d["sim"][32 * o:32 * o + K, :],
                                     start=True, stop=True,
                                     tile_position=(32 * o, 0))
                    # relu(x + b): split between DVE and ACT to balance B2
                    if o % 2 == 0:
                        nc.vector.tensor_scalar(d["ctx2"][:, o, :], ps[:],
                                                bu[:, o:o + 1], 0.0,
                                                mybir.AluOpType.add,
                                                mybir.AluOpType.max)
                    else:
                        nc.scalar.activation(d["ctx2"][:, o, :], ps[:], AF.Relu,
                                             bias=bu[:, o:o + 1], scale=1.0)

            def final(t, orange):
                d = st[t]
                if "ot" not in d:
                    d["ot"] = bp.tile([128, 4, 512], bf16, tag="out", bufs=2,
                                      name="ot")
                ot = d["ot"]
                korder = [4, 5, 6, 7, 0, 1, 2, 3]
                for o in orange:
                    ps = ppM.tile([128, 512], f32, tag="mm")
                    for i, k in enumerate(korder):
                        rhs = (d["ctx2"][:, k, :] if k < 4
                               else Fres[:, t, (k - 4) * 512:(k - 3) * 512])
                        nc.tensor.matmul(ps[:], wf[:, k, o, :], rhs,
                                         start=(i == 0), stop=(i == 7))
                    if o % 2 == 0:
                        nc.vector.tensor_scalar(ot[:, o, :], ps[:],
                                                bf[:, o:o + 1], 0.0,
                                                mybir.AluOpType.add,
                                                mybir.AluOpType.max)
                    else:
                        nc.scalar.activation(ot[:, o, :], ps[:], AF.Relu,
                                             bias=bf[:, o:o + 1], scale=1.0)
                if orange[-1] == 3:
                    nc.sync.dma_start(
                        out_d[:, t * TS:(t + 1) * TS],
                        ot[:].rearrange("p a s -> p (a s)"))
                    st[t] = None

            # attention ramp for tiles 0..AH-1 (wavefront order)
            stages = [attA, attB, attC, attD]
            for s in range(AH + 3):
                for k, f in enumerate(stages):
                    t = s - k
                    if 0 <= t < AH:
                        f(t)

            # per-step order: the exp/recip latencies after attA/attB need
            # >1us of PE work between the attention stages -- the final-conv
            # chains provide it (attB only ~0.5us after attA stalled ~0.7us
            # per tile in earlier layouts)
            for t in range(NT):
                ta = t + AH
                if ta < NT:
                    attA(ta)
                ctx2f(t, (0, 1))
                ctx2f(t, (2, 3))
                final(t, (0, 1))
                if ta < NT:
                    attB(ta)
                final(t, (2,))
                if ta < NT:
                    attC(ta)
                final(t, (3,))
                if ta < NT:
                    attD(ta)

    nc.compile()
    return nc


def _fold(w, b, s, t):
    """conv+BN fold: y = s*(Wx+b)+t = (s.W)x + (s*b+t)."""
    w = np.asarray(w, np.float32)
    b = np.asarray(b, np.float32)
    s = np.asarray(s, np.float32)
    t = np.asarray(t, np.float32)
    return (s[:, None] * w), (s * b + t)


def _tw(Wmat, kin, kout):
    """[out, in] f32 -> pre-tiled [128, kin*kout*128] bf16 (stationary tiles
    [in-chunk, out-chunk] contiguous per partition)."""
    Wt = np.asarray(Wmat, np.float32).T  # [in, out]
    t = Wt.reshape(kin, 128, kout, 128).transpose(1, 0, 2, 3)
    return np.ascontiguousarray(t.reshape(128, kin * kout * 128)
                                .astype(ml_dtypes.bfloat16))


def _tb(b, nch):
    """[nch*128] f32 -> [128, nch] (partition-major bias tiles)."""
    return np.ascontiguousarray(np.asarray(b, np.float32).reshape(nch, 128).T)


def kernel(feats, probs,
           wp1, bp1, sp1, tp1, wp2, bp2, sp2, tp2,
           wo1, bo1, so1, to1, wo2, bo2, so2, to2,
           wd, bd, sd, td, wu, bu, su, tu,
           wf, bf, sf, tf, clip_num, _trace=False):
    feats = np.asarray(feats, np.float32)
    probs = np.ascontiguousarray(np.asarray(probs, np.float32))

    W1, B1 = _fold(wp1, bp1, sp1, tp1)
    W2, B2 = _fold(wp2, bp2, sp2, tp2)
    WO1, BO1 = _fold(wo1, bo1, so1, to1)
    WO2, BO2 = _fold(wo2, bo2, so2, to2)
    WD, BD = _fold(wd, bd, sd, td)
    WU, BU = _fold(wu, bu, su, tu)
    WF, BF = _fold(wf, bf, sf, tf)

    shared = {
        "wp1b": _tw(W1, 4, 2), "bp1": _tb(B1, 2),
        "wp2b": _tw(W2, 2, 2), "bp2": _tb(B2, 2),
        "wo1b": _tw(WO1, 4, 2), "bo1": _tb(BO1, 2),
        "wo2b": _tw(WO2, 2, 2), "bo2": _tb(BO2, 2),
        "wdb": _tw(WD, 4, 2), "bd": _tb(BD, 2),
        "wub": _tw(WU, 2, 4), "bu": _tb(BU, 4),
        "wfb": _tw(WF, 8, 4), "bf": _tb(BF, 4),
        "ident": np.eye(128, dtype=np.float32),
        "ones": np.ones((128, 32), np.float32),
    }

    fb = np.asarray(feats.reshape(N, C, HW), np.float32).astype(ml_dtypes.bfloat16)
    pr = probs.reshape(N, K, HW)
    in_maps = []
    for j in range(8):
        n, h = j // 2, j % 2
        sl = slice(h * HALF, (h + 1) * HALF)
        fh = fb[n, :, sl]
        # channel-major tiles: (p, t, a, s) = F[a*128+p, t*512+s]
        fT = fh.reshape(4, 128, NT, 512).transpose(1, 2, 0, 3)
        # spatial-major (pre-transposed): (p, tt, c) = F[c, tt*128+p]
        fS = np.ascontiguousarray(fh.T).reshape(NCH, 128, C).transpose(1, 0, 2)
        # probsT: (p, tt, k) = probs[k, tt*128+p], col 19 = -100 -> exp = 0
        ph = pr[n, :, sl]
        pt = np.full((HALF, KP), -100.0, np.float32)
        pt[:, :K] = ph.T
        pt = pt.reshape(NCH, 128, KP).transpose(1, 0, 2)
        in_maps.append({
            "featsT": np.ascontiguousarray(fT.reshape(128, NT * TS)),
            "featsS": np.ascontiguousarray(fS.reshape(128, NCH * C)),
            "probsT": np.ascontiguousarray(pt.reshape(128, NCH * KP)),
            **shared,
        })

    if "nc" not in _CACHED:
        _CACHED["nc"] = _build_nc()
    nc = _CACHED["nc"]

    res = run_bass_kernel_spmd(nc, in_maps, list(range(8)), trace=_trace)
    out = np.empty((N, OUT, HW), np.float32)
    for j in range(8):
        n, h = j // 2, j % 2
        o = np.asarray(res.results[j]["outT"], np.float32).reshape(128, NT, 4, 512)
        out[n, :, h * HALF:(h + 1) * HALF] = (
            o.transpose(2, 0, 1, 3).reshape(OUT, HALF))
    if _trace:
        kernel.last_exec_time_ns = res.exec_time_ns
        kernel.last_results = res
    return out.reshape(N, OUT, H, W)
